# revision 7
# baseline (speedup 1.0000x reference)
"""Trainium2 kernel for nn_EquivariantConvExp (dense_cnn, memory-bound).

The reference applies, per block i, a truncated conv-exponential
exp(Conv_i) to z of shape [B, 1, 1, 2].  A 3x3 "same" conv on a 1x2 image
only ever uses the middle filter row, so Conv_i acts on each sample pair
(z0, z1) as the 2x2 matrix A_i = [[f11, f12], [f10, f11]] built from the
expanded filter's middle row.  The truncated exponential is the 2x2 matrix
E_i = sum_{k=0}^{10} A_i^k / k!, and the chain over 8 blocks collapses to a
single 2x2 matrix M = E_0 @ E_1 @ ... @ E_7 with z_out = z_in @ M^T.
log|det| is the per-sample constant c = -2 * sum_i f11_i.

The device kernel is therefore a streaming 2x2 affine map over 4M pairs,
data-parallel across 8 NeuronCores (batch sharding, no collectives), plus a
constant fill for the logdet plane.  All DMA transfers are fully contiguous;
the pair deinterleave happens in the compute engines via stride-2 access
patterns (ACT scales the four terms, DVE adds them pairwise, writing the
interleaved output tile directly).

Raw bass (manual semaphores) rather than Tile: this walrus build encodes at
most one attached sync-wait per instruction, which Tile's auto-generated
multi-wait sync_info exceeds; standalone wait_ge instructions are the
proven-good encoding.
"""

import numpy as np

N_CORES = 8
P = 128                      # SBUF partitions
T = 4096                     # fp32 elements per partition per tile
B = 4194304                  # total batch rows
BS = B // N_CORES            # rows per core
FLAT = BS * 2                # fp32 elements per core
NT = FLAT // (P * T)         # tiles per core
TH = T // 2                  # pairs per partition per tile
LD_W = BS // P               # logdet elements per partition

N_TERMS = 10

# Set by the test harness to capture a profile; LAST_RESULT holds the
# BassKernelResults of the most recent device run.
TRACE = False
LAST_RESULT = None


def _collapse(weights, basis):
    """Reduce (weights, basis) to the 2x2 matrix M and logdet constant c."""
    w = np.asarray(weights, dtype=np.float64)
    bs = np.asarray(basis, dtype=np.float64)
    n_blocks = w.shape[0]
    M = np.eye(2)
    c = 0.0
    for i in range(n_blocks):
        f = np.einsum("n,nhw->hw", w[i], bs[i, :, 0, 0])
        A = np.array([[f[1, 1], f[1, 2]], [f[1, 0], f[1, 1]]])
        E = np.eye(2)
        term = np.eye(2)
        for k in range(1, N_TERMS + 1):
            term = term @ A / k
            E = E + term
        M = M @ E
        c -= 2.0 * f[1, 1]
    return M, c


def _build(nc_cls, mybir, M, c):
    f32 = mybir.dt.float32
    AF = mybir.ActivationFunctionType
    OP = mybir.AluOpType

    m00 = float(np.float32(M[0, 0]))
    m01 = float(np.float32(M[0, 1]))
    m10 = float(np.float32(M[1, 0]))
    m11 = float(np.float32(M[1, 1]))
    cf = float(np.float32(c))

    nc = nc_cls()
    xin = nc.declare_dram_parameter("x", [BS, 2], f32, isOutput=False)
    zout = nc.declare_dram_parameter("z", [BS, 2], f32, isOutput=True)
    ldout = nc.declare_dram_parameter("ld", [BS], f32, isOutput=True)

    xv = xin[:].flatten().rearrange("(n p t) -> n p t", p=P, t=T)
    zv = zout[:].flatten().rearrange("(n p t) -> n p t", p=P, t=T)
    ldv = ldout[:].rearrange("(p t) -> p t", p=P)

    with (
        nc.sbuf_tensor([P, NT * T], f32) as xbuf,
        nc.sbuf_tensor([P, NT * T], f32) as obuf,
        nc.sbuf_tensor([P, NT * 4 * TH], f32) as tbuf,
        nc.sbuf_tensor([P, LD_W], f32) as lt,
        nc.semaphore("in_sem") as in_sem,
        nc.semaphore("out_sem") as out_sem,
        nc.semaphore("ld_sem") as ld_sem,
        nc.semaphore("act_sem") as act_sem,
        nc.semaphore("dve_sem") as dve_sem,
        nc.semaphore("pool_sem") as pool_sem,
        nc.Block() as block,
    ):

        @block.gpsimd
        def _(g):
            g.memset(lt[:], cf).then_inc(pool_sem, 1)

        @block.sync
        def _(sync):
            for n in range(NT):
                sync.dma_start(
                    out=xbuf[:, n * T : (n + 1) * T], in_=xv[n]
                ).then_inc(in_sem, 16)
            for n in range(NT):
                sync.wait_ge(dve_sem, 2 * (n + 1))
                sync.dma_start(
                    out=zv[n], in_=obuf[:, n * T : (n + 1) * T]
                ).then_inc(out_sem, 16)
            sync.wait_ge(out_sem, 16 * NT)
            sync.wait_ge(ld_sem, 16)

        @block.scalar
        def _(s):
            # logdet store rides the ACT HWDGE ring so it overlaps the
            # x loads on the SP ring.
            s.wait_ge(pool_sem, 1)
            s.dma_start(out=ldv[:, :], in_=lt[:]).then_inc(ld_sem, 16)
            for n in range(NT):
                s.wait_ge(in_sem, 16 * (n + 1))
                xt = xbuf[:, n * T : (n + 1) * T]
                xe = xt[:, 0::2]
                xo = xt[:, 1::2]
                tb0 = n * 4 * TH
                s.activation(
                    tbuf[:, tb0 + 0 * TH : tb0 + 1 * TH], xe, AF.Copy,
                    bias=0.0, scale=m00,
                ).then_inc(act_sem, 1)
                s.activation(
                    tbuf[:, tb0 + 1 * TH : tb0 + 2 * TH], xo, AF.Copy,
                    bias=0.0, scale=m01,
                ).then_inc(act_sem, 1)
                s.activation(
                    tbuf[:, tb0 + 2 * TH : tb0 + 3 * TH], xe, AF.Copy,
                    bias=0.0, scale=m10,
                ).then_inc(act_sem, 1)
                s.activation(
                    tbuf[:, tb0 + 3 * TH : tb0 + 4 * TH], xo, AF.Copy,
                    bias=0.0, scale=m11,
                ).then_inc(act_sem, 1)

        @block.vector
        def _(v):
            for n in range(NT):
                v.wait_ge(act_sem, 4 * (n + 1))
                ot = obuf[:, n * T : (n + 1) * T]
                oe = ot[:, 0::2]
                oo = ot[:, 1::2]
                tb0 = n * 4 * TH
                v.tensor_tensor(
                    oe,
                    tbuf[:, tb0 + 0 * TH : tb0 + 1 * TH],
                    tbuf[:, tb0 + 1 * TH : tb0 + 2 * TH],
                    OP.add,
                ).then_inc(dve_sem, 1)
                v.tensor_tensor(
                    oo,
                    tbuf[:, tb0 + 2 * TH : tb0 + 3 * TH],
                    tbuf[:, tb0 + 3 * TH : tb0 + 4 * TH],
                    OP.add,
                ).then_inc(dve_sem, 1)

    return nc


def _run_device(x, M, c):
    import concourse.bass as bass
    import concourse.mybir as mybir
    from concourse.bass_utils import run_bass_kernel_spmd

    nc = _build(bass.Bass, mybir, M, c)

    xs = np.ascontiguousarray(x.reshape(N_CORES, BS, 2))
    in_maps = [{"x": xs[i]} for i in range(N_CORES)]
    res = run_bass_kernel_spmd(
        nc, in_maps, core_ids=list(range(N_CORES)), trace=TRACE
    )
    global LAST_RESULT
    LAST_RESULT = res
    z = np.concatenate([res.results[i]["z"] for i in range(N_CORES)], axis=0)
    ld = np.concatenate([res.results[i]["ld"] for i in range(N_CORES)], axis=0)
    return z, ld


def kernel(x, weights, basis):
    x = np.ascontiguousarray(np.asarray(x, dtype=np.float32))
    M, c = _collapse(weights, basis)
    if x.shape != (B, 2):
        # Shape drift guard: same math on host.
        z = (x.astype(np.float64) @ M.T).astype(np.float32)
        ld = np.full((x.shape[0],), np.float32(c), dtype=np.float32)
        return z, ld
    return _run_device(x, M, c)


# revision 35
# speedup vs baseline: 1.1377x; 1.1377x over previous
"""Trainium2 kernel for nn_EquivariantConvExp (dense_cnn, memory-bound).

The reference applies, per block i, a truncated conv-exponential
exp(Conv_i) to z of shape [B, 1, 1, 2].  A 3x3 "same" conv on a 1x2 image
only ever uses the middle filter row, so Conv_i acts on each sample pair
(z0, z1) as the 2x2 matrix A_i = [[f11, f12], [f10, f11]] built from the
expanded filter's middle row.  The truncated exponential is the 2x2 matrix
E_i = sum_{k=0}^{10} A_i^k / k!, and the chain over 8 blocks collapses to a
single 2x2 matrix M = E_0 @ E_1 @ ... @ E_7 with z_out = z_in @ M^T.
log|det| is the per-sample constant c = -2 * sum_i f11_i.

The device kernel is therefore a streaming 2x2 affine map over 4M pairs,
data-parallel across 8 NeuronCores (batch sharding, no collectives), plus a
constant fill for the logdet plane.  All DMA transfers are fully contiguous;
the pair deinterleave happens in the compute engines via stride-2 access
patterns (ACT scales the four terms, DVE adds them pairwise, writing the
interleaved output tile directly).

Raw bass (manual semaphores) rather than Tile: this walrus build encodes at
most one attached sync-wait per instruction, which Tile's auto-generated
multi-wait sync_info exceeds; standalone wait_ge instructions are the
proven-good encoding.
"""

import numpy as np

N_CORES = 8
P = 128                      # SBUF partitions
B = 4194304                  # total batch rows
BS = B // N_CORES            # rows per core
FLAT = BS * 2                # fp32 elements per core
W = FLAT // P                # total columns per partition
T = 2048                     # cols per DMA tile (1 MiB transfers)
NT = W // T                  # DMA tiles (loads and stores each)
CW = 1024                    # cols per compute chunk
K = T // CW                  # compute chunks per DMA tile
NCH = NT * K                 # total compute chunks
CH = CW // 2                 # pairs per partition per chunk
LD_W = BS // P               # logdet elements per partition

N_TERMS = 10

# Set by the test harness to capture a profile; LAST_RESULT holds the
# BassKernelResults of the most recent device run.
TRACE = False
LAST_RESULT = None


def _collapse(weights, basis):
    """Reduce (weights, basis) to the 2x2 matrix M and logdet constant c."""
    w = np.asarray(weights, dtype=np.float64)
    bs = np.asarray(basis, dtype=np.float64)
    n_blocks = w.shape[0]
    M = np.eye(2)
    c = 0.0
    for i in range(n_blocks):
        f = np.einsum("n,nhw->hw", w[i], bs[i, :, 0, 0])
        A = np.array([[f[1, 1], f[1, 2]], [f[1, 0], f[1, 1]]])
        E = np.eye(2)
        term = np.eye(2)
        for k in range(1, N_TERMS + 1):
            term = term @ A / k
            E = E + term
        M = M @ E
        c -= 2.0 * f[1, 1]
    return M, c


def _build(nc_cls, mybir, M, c):
    f32 = mybir.dt.float32
    AF = mybir.ActivationFunctionType
    OP = mybir.AluOpType

    m00 = float(np.float32(M[0, 0]))
    m01 = float(np.float32(M[0, 1]))
    m10 = float(np.float32(M[1, 0]))
    m11 = float(np.float32(M[1, 1]))
    cf = float(np.float32(c))

    nc = nc_cls()
    xin = nc.declare_dram_parameter("x", [BS, 2], f32, isOutput=False)
    zout = nc.declare_dram_parameter("z", [BS, 2], f32, isOutput=True)
    ldout = nc.declare_dram_parameter("ld", [BS], f32, isOutput=True)

    # Tile-major views: DMA tile n is the contiguous flat range
    # [n*P*T, (n+1)*P*T), laid out [P, T] with partition p owning a
    # contiguous T-element run.  Every transfer is a fully sequential
    # DRAM range with 16 KiB-per-partition chunks (the fastest observed
    # DMA pattern); compute sub-chunks tiles at CW columns, so pipeline
    # granularity stays fine without shrinking transfers.
    xv = xin[:].flatten().rearrange("(n p t) -> n p t", p=P, t=T)
    zv = zout[:].flatten().rearrange("(n p t) -> n p t", p=P, t=T)
    ldv = ldout[:].rearrange("(p t) -> p t", p=P)

    from contextlib import ExitStack

    with ExitStack() as ctx:
        xbuf = ctx.enter_context(nc.sbuf_tensor([P, W], f32))
        obuf = ctx.enter_context(nc.sbuf_tensor([P, W], f32))
        tbuf = ctx.enter_context(nc.sbuf_tensor([P, W], f32))
        lt = ctx.enter_context(nc.sbuf_tensor([P, LD_W], f32))
        # One semaphore per load: the 16 SDMA engine-slots increment
        # independently and can skew across outstanding transfers, so a
        # shared counter's cumulative value does not imply transfer n
        # fully landed.  A per-transfer sem hitting 16 does.
        in_sems = [
            ctx.enter_context(nc.semaphore(f"in_sem{n}")) for n in range(NT)
        ]
        out_sem = ctx.enter_context(nc.semaphore("out_sem"))
        act_sem = ctx.enter_context(nc.semaphore("act_sem"))
        pool_sem = ctx.enter_context(nc.semaphore("pool_sem"))
        dve_sem = ctx.enter_context(nc.semaphore("dve_sem"))
        block = ctx.enter_context(nc.Block())

        @block.gpsimd
        def _(g):
            g.memset(lt[:], cf)
            g.drain().then_inc(pool_sem, 1)

        @block.sync
        def _(sync):
            # SP ring: read stream back-to-back, then the write stream as
            # each tile's compute lands.  The logdet plane rides the ACT
            # HWDGE ring concurrently.
            for n in range(NT):
                sync.dma_start(
                    out=xbuf[:, n * T : (n + 1) * T], in_=xv[n]
                ).then_inc(in_sems[n], 16)
            for n in range(NT):
                sync.wait_ge(dve_sem, K * (n + 1))
                sync.dma_start(
                    out=zv[n], in_=obuf[:, n * T : (n + 1) * T]
                ).then_inc(out_sem, 16)
            sync.wait_ge(out_sem, 16 * (NT + 1))

        @block.scalar
        def _(s):
            # logdet store on the ACT ring (overlaps the loads), then the
            # per-tile cross terms tb = xo*m01, tc = xe*m10 (half-width,
            # stride-2 reads).
            s.wait_ge(pool_sem, 1)
            s.dma_start(out=ldv[:, :], in_=lt[:]).then_inc(out_sem, 16)
            for j in range(NCH):
                if j % K == 0:
                    s.wait_ge(in_sems[j // K], 16)
                xt = xbuf[:, j * CW : (j + 1) * CW]
                xe = xt[:, 0::2]
                xo = xt[:, 1::2]
                tb0 = j * CW
                s.activation(
                    tbuf[:, tb0 : tb0 + CH], xo, AF.Copy,
                    bias=0.0, scale=m01,
                )
                s.activation(
                    tbuf[:, tb0 + CH : tb0 + 2 * CH], xe, AF.Copy,
                    bias=0.0, scale=m10,
                )
                s.drain().then_inc(act_sem, 1)

        @block.vector
        def _(v):
            # oe = xe*m00 + tb ; oo = xo*m11 + tc  (fused on DVE)
            for j in range(NCH):
                v.wait_ge(act_sem, j + 1)
                xt = xbuf[:, j * CW : (j + 1) * CW]
                xe = xt[:, 0::2]
                xo = xt[:, 1::2]
                ot = obuf[:, j * CW : (j + 1) * CW]
                oe = ot[:, 0::2]
                oo = ot[:, 1::2]
                tb0 = j * CW
                v.scalar_tensor_tensor(
                    oe, xe, m00, tbuf[:, tb0 : tb0 + CH],
                    OP.mult, OP.add,
                )
                v.scalar_tensor_tensor(
                    oo, xo, m11, tbuf[:, tb0 + CH : tb0 + 2 * CH],
                    OP.mult, OP.add,
                )
                v.drain().then_inc(dve_sem, 1)

    return nc


def _run_device(x, M, c):
    import concourse.bass as bass
    import concourse.mybir as mybir
    from concourse.bass_utils import run_bass_kernel_spmd

    nc = _build(bass.Bass, mybir, M, c)

    xs = np.ascontiguousarray(x.reshape(N_CORES, BS, 2))
    in_maps = [{"x": xs[i]} for i in range(N_CORES)]
    res = run_bass_kernel_spmd(
        nc, in_maps, core_ids=list(range(N_CORES)), trace=TRACE
    )
    global LAST_RESULT
    LAST_RESULT = res
    z = np.concatenate([res.results[i]["z"] for i in range(N_CORES)], axis=0)
    ld = np.concatenate([res.results[i]["ld"] for i in range(N_CORES)], axis=0)
    return z, ld


def kernel(x, weights, basis):
    x = np.ascontiguousarray(np.asarray(x, dtype=np.float32))
    M, c = _collapse(weights, basis)
    if x.shape != (B, 2):
        # Shape drift guard: same math on host.
        z = (x.astype(np.float64) @ M.T).astype(np.float32)
        ld = np.full((x.shape[0],), np.float32(c), dtype=np.float32)
        return z, ld
    return _run_device(x, M, c)


# revision 36
# speedup vs baseline: 1.2455x; 1.0947x over previous
"""Trainium2 kernel for nn_EquivariantConvExp (dense_cnn, memory-bound).

The reference applies, per block i, a truncated conv-exponential
exp(Conv_i) to z of shape [B, 1, 1, 2].  A 3x3 "same" conv on a 1x2 image
only ever uses the middle filter row, so Conv_i acts on each sample pair
(z0, z1) as the 2x2 matrix A_i = [[f11, f12], [f10, f11]] built from the
expanded filter's middle row.  The truncated exponential is the 2x2 matrix
E_i = sum_{k=0}^{10} A_i^k / k!, and the chain over 8 blocks collapses to a
single 2x2 matrix M = E_0 @ E_1 @ ... @ E_7 with z_out = z_in @ M^T.
log|det| is the per-sample constant c = -2 * sum_i f11_i.

The device kernel is therefore a streaming 2x2 affine map over 4M pairs,
data-parallel across 8 NeuronCores (batch sharding, no collectives), plus a
constant fill for the logdet plane.  Per core: 4 MiB in + 6 MiB out at the
~360 GB/s per-NC HBM cap -> ~27 us of streaming, plus ~7 us of fixed NRT/
preamble cost; measured ~37-43 us end-to-end.

Structure per core:
  - SP HWDGE ring: four 1 MiB contiguous loads back-to-back, then four
    1 MiB stores, each issued as its tile's compute lands (paced by
    dve_sem).  The logdet plane rides the ACT HWDGE ring so it streams
    concurrently with the loads.
  - ACT: per 1024-col chunk, the two cross terms tb = xo*m01 and
    tc = xe*m10 (stride-2 reads; ScalarE has no fp32 fast mode, ~1.2
    ns/elem regardless of stride).
  - DVE: per chunk, the two fused scalar_tensor_tensor ops
    oe = xe*m00 + tb and oo = xo*m11 + tc, writing the interleaved
    output tile in place.
  - Pool: memset of the logdet constant only (its elementwise compute is
    ~100x too slow for strided work, and SWDGE adds exit-drain cost).

Raw bass (manual semaphores) rather than Tile: this walrus build encodes at
most one attached sync-wait per instruction, which Tile's auto-generated
multi-wait sync_info exceeds; standalone wait_ge instructions are the
proven-good encoding.  Two hard-won correctness rules: (1) cross-engine
handoffs must signal via drain().then_inc — a bare op.then_inc can fire
before the producer's SBUF writes are visible to a DMA reader; (2) each
load gets its own semaphore — the 16 SDMA engine-slots increment
independently and skew across outstanding transfers, so a shared counter's
cumulative value does not prove an individual transfer landed.
"""

import numpy as np

N_CORES = 8
P = 128                      # SBUF partitions
B = 4194304                  # total batch rows
BS = B // N_CORES            # rows per core
FLAT = BS * 2                # fp32 elements per core
W = FLAT // P                # total columns per partition
T = 2048                     # cols per DMA tile (1 MiB transfers)
NT = W // T                  # DMA tiles (loads and stores each)
CW = 1024                    # cols per compute chunk
K = T // CW                  # compute chunks per DMA tile
NCH = NT * K                 # total compute chunks
CH = CW // 2                 # pairs per partition per chunk
LD_W = BS // P               # logdet elements per partition

N_TERMS = 10

# Set by the test harness to capture a profile; LAST_RESULT holds the
# BassKernelResults of the most recent device run.
TRACE = False
LAST_RESULT = None


def _collapse(weights, basis):
    """Reduce (weights, basis) to the 2x2 matrix M and logdet constant c."""
    w = np.asarray(weights, dtype=np.float64)
    bs = np.asarray(basis, dtype=np.float64)
    n_blocks = w.shape[0]
    M = np.eye(2)
    c = 0.0
    for i in range(n_blocks):
        f = np.einsum("n,nhw->hw", w[i], bs[i, :, 0, 0])
        A = np.array([[f[1, 1], f[1, 2]], [f[1, 0], f[1, 1]]])
        E = np.eye(2)
        term = np.eye(2)
        for k in range(1, N_TERMS + 1):
            term = term @ A / k
            E = E + term
        M = M @ E
        c -= 2.0 * f[1, 1]
    return M, c


def _build(nc_cls, mybir, M, c):
    f32 = mybir.dt.float32
    AF = mybir.ActivationFunctionType
    OP = mybir.AluOpType

    m00 = float(np.float32(M[0, 0]))
    m01 = float(np.float32(M[0, 1]))
    m10 = float(np.float32(M[1, 0]))
    m11 = float(np.float32(M[1, 1]))
    cf = float(np.float32(c))

    nc = nc_cls()
    xin = nc.declare_dram_parameter("x", [BS, 2], f32, isOutput=False)
    zout = nc.declare_dram_parameter("z", [BS, 2], f32, isOutput=True)
    ldout = nc.declare_dram_parameter("ld", [BS], f32, isOutput=True)

    # Tile-major views: DMA tile n is the contiguous flat range
    # [n*P*T, (n+1)*P*T), laid out [P, T] with partition p owning a
    # contiguous T-element run.  Every transfer is a fully sequential
    # DRAM range with 16 KiB-per-partition chunks (the fastest observed
    # DMA pattern); compute sub-chunks tiles at CW columns, so pipeline
    # granularity stays fine without shrinking transfers.
    xv = xin[:].flatten().rearrange("(n p t) -> n p t", p=P, t=T)
    zv = zout[:].flatten().rearrange("(n p t) -> n p t", p=P, t=T)
    ldv = ldout[:].rearrange("(p t) -> p t", p=P)

    from contextlib import ExitStack

    with ExitStack() as ctx:
        xbuf = ctx.enter_context(nc.sbuf_tensor([P, W], f32))
        obuf = ctx.enter_context(nc.sbuf_tensor([P, W], f32))
        tbuf = ctx.enter_context(nc.sbuf_tensor([P, W], f32))
        lt = ctx.enter_context(nc.sbuf_tensor([P, LD_W], f32))
        # One semaphore per load: the 16 SDMA engine-slots increment
        # independently and can skew across outstanding transfers, so a
        # shared counter's cumulative value does not imply transfer n
        # fully landed.  A per-transfer sem hitting 16 does.
        in_sems = [
            ctx.enter_context(nc.semaphore(f"in_sem{n}")) for n in range(NT)
        ]
        out_sem = ctx.enter_context(nc.semaphore("out_sem"))
        act_sem = ctx.enter_context(nc.semaphore("act_sem"))
        pool_sem = ctx.enter_context(nc.semaphore("pool_sem"))
        dve_sem = ctx.enter_context(nc.semaphore("dve_sem"))
        block = ctx.enter_context(nc.Block())

        @block.gpsimd
        def _(g):
            g.memset(lt[:], cf)
            g.drain().then_inc(pool_sem, 1)

        @block.sync
        def _(sync):
            # SP ring: read stream back-to-back, then the write stream as
            # each tile's compute lands.  The logdet plane rides the ACT
            # HWDGE ring concurrently.
            for n in range(NT):
                sync.dma_start(
                    out=xbuf[:, n * T : (n + 1) * T], in_=xv[n]
                ).then_inc(in_sems[n], 16)
            for n in range(NT):
                sync.wait_ge(dve_sem, K * (n + 1))
                sync.dma_start(
                    out=zv[n], in_=obuf[:, n * T : (n + 1) * T]
                ).then_inc(out_sem, 16)
            sync.wait_ge(out_sem, 16 * (NT + 1))

        @block.scalar
        def _(s):
            # logdet store on the ACT ring (overlaps the loads), then the
            # per-tile cross terms tb = xo*m01, tc = xe*m10 (half-width,
            # stride-2 reads).
            s.wait_ge(pool_sem, 1)
            s.dma_start(out=ldv[:, :], in_=lt[:]).then_inc(out_sem, 16)
            for j in range(NCH):
                if j % K == 0:
                    s.wait_ge(in_sems[j // K], 16)
                xt = xbuf[:, j * CW : (j + 1) * CW]
                xe = xt[:, 0::2]
                xo = xt[:, 1::2]
                tb0 = j * CW
                s.activation(
                    tbuf[:, tb0 : tb0 + CH], xo, AF.Copy,
                    bias=0.0, scale=m01,
                )
                s.activation(
                    tbuf[:, tb0 + CH : tb0 + 2 * CH], xe, AF.Copy,
                    bias=0.0, scale=m10,
                )
                s.drain().then_inc(act_sem, 1)

        @block.vector
        def _(v):
            # oe = xe*m00 + tb ; oo = xo*m11 + tc  (fused on DVE)
            for j in range(NCH):
                v.wait_ge(act_sem, j + 1)
                xt = xbuf[:, j * CW : (j + 1) * CW]
                xe = xt[:, 0::2]
                xo = xt[:, 1::2]
                ot = obuf[:, j * CW : (j + 1) * CW]
                oe = ot[:, 0::2]
                oo = ot[:, 1::2]
                tb0 = j * CW
                v.scalar_tensor_tensor(
                    oe, xe, m00, tbuf[:, tb0 : tb0 + CH],
                    OP.mult, OP.add,
                )
                v.scalar_tensor_tensor(
                    oo, xo, m11, tbuf[:, tb0 + CH : tb0 + 2 * CH],
                    OP.mult, OP.add,
                )
                v.drain().then_inc(dve_sem, 1)

    return nc


def _run_device(x, M, c):
    import concourse.bass as bass
    import concourse.mybir as mybir
    from concourse.bass_utils import run_bass_kernel_spmd

    nc = _build(bass.Bass, mybir, M, c)

    xs = np.ascontiguousarray(x.reshape(N_CORES, BS, 2))
    in_maps = [{"x": xs[i]} for i in range(N_CORES)]
    res = run_bass_kernel_spmd(
        nc, in_maps, core_ids=list(range(N_CORES)), trace=TRACE
    )
    global LAST_RESULT
    LAST_RESULT = res
    z = np.concatenate([res.results[i]["z"] for i in range(N_CORES)], axis=0)
    ld = np.concatenate([res.results[i]["ld"] for i in range(N_CORES)], axis=0)
    return z, ld


def kernel(x, weights, basis):
    x = np.ascontiguousarray(np.asarray(x, dtype=np.float32))
    M, c = _collapse(weights, basis)
    if x.shape != (B, 2):
        # Shape drift guard: same math on host.
        z = (x.astype(np.float64) @ M.T).astype(np.float32)
        ld = np.full((x.shape[0],), np.float32(c), dtype=np.float32)
        return z, ld
    return _run_device(x, M, c)


# revision 38
# speedup vs baseline: 1.2504x; 1.0040x over previous
"""Trainium2 kernel for nn_EquivariantConvExp (dense_cnn, memory-bound).

The reference applies, per block i, a truncated conv-exponential
exp(Conv_i) to z of shape [B, 1, 1, 2].  A 3x3 "same" conv on a 1x2 image
only ever uses the middle filter row, so Conv_i acts on each sample pair
(z0, z1) as the 2x2 matrix A_i = [[f11, f12], [f10, f11]] built from the
expanded filter's middle row.  The truncated exponential is the 2x2 matrix
E_i = sum_{k=0}^{10} A_i^k / k!, and the chain over 8 blocks collapses to a
single 2x2 matrix M = E_0 @ E_1 @ ... @ E_7 with z_out = z_in @ M^T.
log|det| is the per-sample constant c = -2 * sum_i f11_i.

The device kernel is therefore a streaming 2x2 affine map over 4M pairs,
data-parallel across 8 NeuronCores (batch sharding, no collectives), plus a
constant fill for the logdet plane.  Per core: 4 MiB in + 6 MiB out at the
~360 GB/s per-NC HBM cap -> ~27 us of streaming, plus ~7 us of fixed NRT/
preamble cost; measured ~37-43 us end-to-end.

Structure per core:
  - SP HWDGE ring: four 1 MiB contiguous loads back-to-back, then four
    1 MiB stores, each issued as its tile's compute lands (paced by
    dve_sem).  The logdet plane rides the ACT HWDGE ring so it streams
    concurrently with the loads.
  - ACT: per 1024-col chunk, the two cross terms tb = xo*m01 and
    tc = xe*m10 (stride-2 reads; ScalarE has no fp32 fast mode, ~1.2
    ns/elem regardless of stride).
  - DVE: per chunk, the two fused scalar_tensor_tensor ops
    oe = xe*m00 + tb and oo = xo*m11 + tc, writing the interleaved
    output tile in place.
  - Pool: memset of the logdet constant only (its elementwise compute is
    ~100x too slow for strided work, and SWDGE adds exit-drain cost).

Raw bass (manual semaphores) rather than Tile: this walrus build encodes at
most one attached sync-wait per instruction, which Tile's auto-generated
multi-wait sync_info exceeds; standalone wait_ge instructions are the
proven-good encoding.  Two hard-won correctness rules: (1) cross-engine
handoffs must signal via drain().then_inc — a bare op.then_inc can fire
before the producer's SBUF writes are visible to a DMA reader; (2) each
load gets its own semaphore — the 16 SDMA engine-slots increment
independently and skew across outstanding transfers, so a shared counter's
cumulative value does not prove an individual transfer landed.
"""

import numpy as np

N_CORES = 8
P = 128                      # SBUF partitions
B = 4194304                  # total batch rows
BS = B // N_CORES            # rows per core
FLAT = BS * 2                # fp32 elements per core
W = FLAT // P                # total columns per partition
T = 2048                     # cols per DMA tile (1 MiB transfers)
NT = W // T                  # DMA tiles (loads and stores each)
CW = 1024                    # cols per compute chunk
K = T // CW                  # compute chunks per DMA tile
NCH = NT * K                 # total compute chunks
CH = CW // 2                 # pairs per partition per chunk
LD_W = BS // P               # logdet elements per partition

N_TERMS = 10

# Set by the test harness to capture a profile; LAST_RESULT holds the
# BassKernelResults of the most recent device run.
TRACE = False
LAST_RESULT = None


def _collapse(weights, basis):
    """Reduce (weights, basis) to the 2x2 matrix M and logdet constant c."""
    w = np.asarray(weights, dtype=np.float64)
    bs = np.asarray(basis, dtype=np.float64)
    n_blocks = w.shape[0]
    M = np.eye(2)
    c = 0.0
    for i in range(n_blocks):
        f = np.einsum("n,nhw->hw", w[i], bs[i, :, 0, 0])
        A = np.array([[f[1, 1], f[1, 2]], [f[1, 0], f[1, 1]]])
        E = np.eye(2)
        term = np.eye(2)
        for k in range(1, N_TERMS + 1):
            term = term @ A / k
            E = E + term
        M = M @ E
        c -= 2.0 * f[1, 1]
    return M, c


def _build(nc_cls, mybir, M, c):
    f32 = mybir.dt.float32
    AF = mybir.ActivationFunctionType
    OP = mybir.AluOpType

    m00 = float(np.float32(M[0, 0]))
    m01 = float(np.float32(M[0, 1]))
    m10 = float(np.float32(M[1, 0]))
    m11 = float(np.float32(M[1, 1]))
    cf = float(np.float32(c))

    nc = nc_cls()
    xin = nc.declare_dram_parameter("x", [BS, 2], f32, isOutput=False)
    zout = nc.declare_dram_parameter("z", [BS, 2], f32, isOutput=True)
    ldout = nc.declare_dram_parameter("ld", [BS], f32, isOutput=True)

    # Tile-major views: DMA tile n is the contiguous flat range
    # [n*P*T, (n+1)*P*T), laid out [P, T] with partition p owning a
    # contiguous T-element run.  Every transfer is a fully sequential
    # DRAM range with 16 KiB-per-partition chunks (the fastest observed
    # DMA pattern); compute sub-chunks tiles at CW columns, so pipeline
    # granularity stays fine without shrinking transfers.
    xv = xin[:].flatten().rearrange("(n p t) -> n p t", p=P, t=T)
    zv = zout[:].flatten().rearrange("(n p t) -> n p t", p=P, t=T)
    ldv = ldout[:].rearrange("(p t) -> p t", p=P)

    from contextlib import ExitStack

    with ExitStack() as ctx:
        xbuf = ctx.enter_context(nc.sbuf_tensor([P, W], f32))
        obuf = ctx.enter_context(nc.sbuf_tensor([P, W], f32))
        tbuf = ctx.enter_context(nc.sbuf_tensor([P, W], f32))
        lt = ctx.enter_context(nc.sbuf_tensor([P, LD_W], f32))
        # One semaphore per load: the 16 SDMA engine-slots increment
        # independently and can skew across outstanding transfers, so a
        # shared counter's cumulative value does not imply transfer n
        # fully landed.  A per-transfer sem hitting 16 does.
        in_sems = [
            ctx.enter_context(nc.semaphore(f"in_sem{n}")) for n in range(NT)
        ]
        out_sem = ctx.enter_context(nc.semaphore("out_sem"))
        act_sem = ctx.enter_context(nc.semaphore("act_sem"))
        pool_sem = ctx.enter_context(nc.semaphore("pool_sem"))
        dve_sem = ctx.enter_context(nc.semaphore("dve_sem"))
        block = ctx.enter_context(nc.Block())

        @block.gpsimd
        def _(g):
            g.memset(lt[:], cf)
            g.drain().then_inc(pool_sem, 1)

        @block.sync
        def _(sync):
            # SP ring: read stream back-to-back, then the write stream as
            # each tile's compute lands.  The logdet plane rides the ACT
            # HWDGE ring concurrently.
            for n in range(NT):
                sync.dma_start(
                    out=xbuf[:, n * T : (n + 1) * T], in_=xv[n]
                ).then_inc(in_sems[n], 16)
            for n in range(NT):
                sync.wait_ge(dve_sem, n + 1)
                sync.dma_start(
                    out=zv[n], in_=obuf[:, n * T : (n + 1) * T]
                ).then_inc(out_sem, 16)
            sync.wait_ge(out_sem, 16 * (NT + 1))

        @block.scalar
        def _(s):
            # logdet store on the ACT ring (overlaps the loads), then the
            # per-tile cross terms tb = xo*m01, tc = xe*m10 (half-width,
            # stride-2 reads).
            s.wait_ge(pool_sem, 1)
            s.dma_start(out=ldv[:, :], in_=lt[:]).then_inc(out_sem, 16)
            for j in range(NCH):
                if j % K == 0:
                    s.wait_ge(in_sems[j // K], 16)
                xt = xbuf[:, j * CW : (j + 1) * CW]
                xe = xt[:, 0::2]
                xo = xt[:, 1::2]
                tb0 = j * CW
                s.activation(
                    tbuf[:, tb0 : tb0 + CH], xo, AF.Copy,
                    bias=0.0, scale=m01,
                )
                s.activation(
                    tbuf[:, tb0 + CH : tb0 + 2 * CH], xe, AF.Copy,
                    bias=0.0, scale=m10,
                )
                s.drain().then_inc(act_sem, 1)

        @block.vector
        def _(v):
            # oe = xe*m00 + tb ; oo = xo*m11 + tc  (fused on DVE).
            # Drain-inc once per TILE (not per chunk): stores only need
            # per-tile granularity, and the saved DVE drains (~0.55 us
            # each) tighten the pipeline under HBM contention.
            for j in range(NCH):
                v.wait_ge(act_sem, j + 1)
                xt = xbuf[:, j * CW : (j + 1) * CW]
                xe = xt[:, 0::2]
                xo = xt[:, 1::2]
                ot = obuf[:, j * CW : (j + 1) * CW]
                oe = ot[:, 0::2]
                oo = ot[:, 1::2]
                tb0 = j * CW
                v.scalar_tensor_tensor(
                    oe, xe, m00, tbuf[:, tb0 : tb0 + CH],
                    OP.mult, OP.add,
                )
                v.scalar_tensor_tensor(
                    oo, xo, m11, tbuf[:, tb0 + CH : tb0 + 2 * CH],
                    OP.mult, OP.add,
                )
                if (j + 1) % K == 0:
                    v.drain().then_inc(dve_sem, 1)

    return nc


def _run_device(x, M, c):
    import concourse.bass as bass
    import concourse.mybir as mybir
    from concourse.bass_utils import run_bass_kernel_spmd

    nc = _build(bass.Bass, mybir, M, c)

    xs = np.ascontiguousarray(x.reshape(N_CORES, BS, 2))
    in_maps = [{"x": xs[i]} for i in range(N_CORES)]
    res = run_bass_kernel_spmd(
        nc, in_maps, core_ids=list(range(N_CORES)), trace=TRACE
    )
    global LAST_RESULT
    LAST_RESULT = res
    z = np.concatenate([res.results[i]["z"] for i in range(N_CORES)], axis=0)
    ld = np.concatenate([res.results[i]["ld"] for i in range(N_CORES)], axis=0)
    return z, ld


def kernel(x, weights, basis):
    x = np.ascontiguousarray(np.asarray(x, dtype=np.float32))
    M, c = _collapse(weights, basis)
    if x.shape != (B, 2):
        # Shape drift guard: same math on host.
        z = (x.astype(np.float64) @ M.T).astype(np.float32)
        ld = np.full((x.shape[0],), np.float32(c), dtype=np.float32)
        return z, ld
    return _run_device(x, M, c)
